# revision 37
# baseline (speedup 1.0000x reference)
"""Trainium2 Bass kernel for nn_Attention_2216203124924 (sparse/varlen GQA attention).

Full computation:
  xq/xk/xv = x @ {wq,wk,wv}.T ; per-head RMSNorm(q,k) ; RoPE via
  rope_cache[positions] ; GQA repeat ; per-segment causal attention
  (segments from cu_seqlens) ; out @ wo.T

Distribution (8 NeuronCores, tensor-parallel over heads):
  core c owns q-heads [4c,4c+4) and kv-head c (GQA groups align),
  wo is row-sharded; each core emits a partial [2048,4096] bf16 output and
  the host sums the 8 partials in f32.

On-device layout is "transposed" ([feature, seq]) so the contraction dim
always sits on SBUF partitions. The kernel is a single software pipeline
over 512-column chunks with NO phase barrier: for each chunk,
qkv projection (o-major: one PSUM accumulator at a time over 32
contraction tiles) -> RMS stats -> RoPE -> per-segment attention ->
output projection, all issued in one stream. The Tile scheduler's
priority order (= issue order) makes the next chunk's projection matmuls
the natural PE filler during the current chunk's attention dependency
stalls, so the PE stays dense and the HAM clock gate never re-throttles.

PE-offloads vs the obvious formulation:
  - RoPE swap-halves via two SBUF->SBUF DMAs (not a PE permute matmul)
  - rsqrt/softmax-denominator broadcasts via gpsimd partition_broadcast
    (not PE ones-outer-product matmuls)
  - attn scale and eps folded into the ACT Sqrt bias/scale
PSUM budget (8 banks): proj acc x2, scores x2, PV acc x1, outproj acc x2,
smalls (stats/transpose/den) x1.

All matmul operands are bf16 (full PE rate). The segment/causal structure
from cu_seqlens and the rope gather by positions are resolved on the host
at build time; the NEFF is specialized to them.
"""

import os
import sys

import numpy as np

for _p in ("/opt/trn_rl_repo",):
    if os.path.isdir(_p) and _p not in sys.path:
        sys.path.insert(0, _p)

S = 2048
D = 4096
HD = 128
HALF = 64
N_HEADS = 32
N_KV = 8
NCORES = 8
QH = N_HEADS // NCORES          # 4 q heads per core
NO = QH + 2                     # projection outputs per core: q0..q3, k, v
DT = D // 128                   # 32 contraction tiles
MC = S // 512                   # 4 m-chunks of 512
NT = S // 128                   # 16 key tiles
GD = 8                          # x d-tiles per DMA group
NG = DT // GD                   # 4 groups per chunk
WG = 4                          # w d-tiles per DMA group
EPS = 1e-6
SCALE = HD ** -0.5

LAST_RESULT = None  # BassKernelResults of the most recent run (for test harness)


def _attention_plan(cu_seqlens):
    """Compile-time mask plan from cu_seqlens.

    Returns (plan, mask_pack):
      plan[mc] = list of (nt, w0, w1, mask_ops); w0/w1 are column offsets
      (multiples of 128, relative to the 512-wide m-chunk) of the contiguous
      valid window; mask_ops = [(j, kind, idx)] for 128-col subtiles needing
      a multiplicative 0/1 mask: kind 'tri' uses a gpsimd affine_select,
      kind 'host' uses mask_pack[:, idx*128:(idx+1)*128].
    """
    idx = np.arange(S)
    seg = np.searchsorted(np.asarray(cu_seqlens), idx, side="right") - 1
    mask_qk = (seg[:, None] == seg[None, :]) & (idx[:, None] >= idx[None, :])
    mask_t = mask_qk.T  # [n, m]

    plan = []
    tiles = []
    tile_ids = {}
    for mc in range(MC):
        entries = []
        for nt in range(NT):
            blk = mask_t[nt * 128:(nt + 1) * 128, mc * 512:(mc + 1) * 512]
            if not blk.any():
                continue
            js = [j for j in range(4) if blk[:, j * 128:(j + 1) * 128].any()]
            jlo, jhi = min(js), max(js)
            assert js == list(range(jlo, jhi + 1)), "valid window not contiguous"
            mops = []
            for j in range(jlo, jhi + 1):
                sub = blk[:, j * 128:(j + 1) * 128]
                if sub.all():
                    continue
                m0g = mc * 512 + j * 128
                n0g = nt * 128
                if m0g == n0g and np.array_equal(
                    sub, idx[:128][None, :] >= idx[:128][:, None]
                ):
                    mops.append((j, "tri", -1))
                else:
                    key = sub.tobytes()
                    if key not in tile_ids:
                        tile_ids[key] = len(tiles)
                        # bias (pre-exp, accumulated into scores PSUM as
                        # biasT.T @ I): lhsT[m, n] = -30 where invalid
                        tiles.append((1.0 - sub.T.astype(np.float32)) * -30.0)
                    mops.append((j, "host", tile_ids[key]))
            entries.append((nt, jlo * 128, (jhi + 1) * 128, mops))
        assert entries, "every query row attends to at least itself"
        plan.append(entries)

    if tiles:
        mask_pack = np.concatenate(tiles, axis=1)
    else:
        mask_pack = np.zeros((128, 128), dtype=np.float32)
    return plan, np.ascontiguousarray(mask_pack)


def _build_graph(plan, n_mask_cols):
    import concourse.bass as bass  # noqa: PLC0415
    import concourse.mybir as mybir  # noqa: PLC0415
    import concourse.tile as tile  # noqa: PLC0415
    from concourse import bacc, bass_isa  # noqa: PLC0415
    from contextlib import ExitStack  # noqa: PLC0415

    f32 = mybir.dt.float32
    bf16 = mybir.dt.bfloat16
    AF = mybir.ActivationFunctionType

    nc = bacc.Bacc()
    # all inputs host-pretiled so every DMA is one contiguous block per
    # partition (large descriptors, few instructions)
    xT_p = nc.declare_dram_parameter("xT", [MC * NG * 128, GD * 512], bf16, isOutput=False)
    wqkv_p = nc.declare_dram_parameter("w_qkv", [NO * 128, DT * 128], bf16, isOutput=False)
    wo_p = nc.declare_dram_parameter("w_o", [4 * 128, QH * D // 4], bf16, isOutput=False)
    cs_p = nc.declare_dram_parameter("cs", [MC * 128, 4 * 512], bf16, isOutput=False)
    constsb_p = nc.declare_dram_parameter(
        "consts_bf", [128, 3 * 128 + n_mask_cols], bf16, isOutput=False
    )
    out_p = nc.declare_dram_parameter("out", [S, D], bf16, isOutput=True)

    with tile.TileContext(nc) as tc, ExitStack() as ctx:
        const = ctx.enter_context(tc.tile_pool(name="const", bufs=1))
        persist = ctx.enter_context(tc.tile_pool(name="persist", bufs=1))

        constsb = const.tile([128, 3 * 128 + n_mask_cols], bf16)
        ones_col_bf = constsb[:, 0:1]
        ident_bf = constsb[:, 128:256]   # identity (for PE transpose + bias MMs)
        tri_bf = constsb[:, 256:384]     # strictly-upper -30 causal bias (lhsT)
        mask_bf = constsb[:, 384:]       # host bias tiles (lhsT), -30 on invalid

        epsq = const.tile([1, 1], f32, name="epsq")
        epsk = const.tile([1, 1], f32, name="epsk")
        nc.gpsimd.memset(epsq[:], HD * EPS)
        nc.gpsimd.memset(epsk[:], EPS)

        # persistent weights + per-chunk K/V tiles
        w_sb = persist.tile([128, NO * DT * 128], bf16)
        wo_sb = persist.tile([128, QH * D], bf16)
        kt_c = [persist.tile([128, 512], bf16, name=f"kt{m}") for m in range(MC)]
        v_c = [persist.tile([128, 512], bf16, name=f"vt{m}") for m in range(MC)]

        # o-pass order: q0, k, v first so attention's head-0 chain can start
        # while q1..q3 are still projecting
        O_ORDER = [0, QH, QH + 1, 1, 2, 3]

        # ---- startup DMAs ----
        # per-o weight blocks in consumption order: each o-pass waits only on
        # its own 1MB slice, not the whole weight stream. wo rides the sync
        # queue AFTER chunk 0's x so it doesn't steal HBM bandwidth from the
        # startup-critical w+x stream (first consumer is outproj(0) at ~75us).
        nc.scalar.dma_start(constsb[:], constsb_p[:])
        for oi, o in enumerate(O_ORDER):
            if o == 0:
                # first pass's weights split 4-ways: the (o0,d0) matmul waits
                # on 256KB instead of 1MB, starting the PE ~6us earlier
                oq = DT * 128 // 4
                for qi in range(4):
                    nc.scalar.dma_start(
                        w_sb[:, qi * oq:(qi + 1) * oq],
                        wqkv_p[0:128, qi * oq:(qi + 1) * oq],
                    )
                continue
            # k/v blocks ride the otherwise-idle gpsimd queue: 3-way DMA
            # spread lands pass 2's weights earlier during the HBM-saturated
            # startup burst (pass k measured a 5.8us wait on this block)
            eng = nc.gpsimd if o >= QH else nc.scalar
            eng.dma_start(
                w_sb[:, o * DT * 128:(o + 1) * DT * 128],
                wqkv_p[o * 128:(o + 1) * 128, :],
            )
        woch = QH * D // 4
        for wci in range(4):
            nc.scalar.dma_start(
                wo_sb[:, wci * woch:(wci + 1) * woch],
                wo_p[wci * 128:(wci + 1) * 128, :],
            )

        # rotating pools
        pxt = ctx.enter_context(tc.tile_pool(name="xstream", bufs=5))
        pcs = ctx.enter_context(tc.tile_pool(name="csstream", bufs=2))
        pqk = ctx.enter_context(tc.tile_pool(name="qkvrot", bufs=2))
        psw = ctx.enter_context(tc.tile_pool(name="swap", bufs=2))
        psq = ctx.enter_context(tc.tile_pool(name="sq", bufs=1))
        prs = ctx.enter_context(tc.tile_pool(name="rs", bufs=2))
        prb = ctx.enter_context(tc.tile_pool(name="rb", bufs=1))
        pt = ctx.enter_context(tc.tile_pool(name="t12", bufs=1))
        pqb = ctx.enter_context(tc.tile_pool(name="qb", bufs=2))
        pex = ctx.enter_context(tc.tile_pool(name="ex", bufs=3))
        pdn = ctx.enter_context(tc.tile_pool(name="dn", bufs=2))
        pat = ctx.enter_context(tc.tile_pool(name="attn", bufs=2))
        pys = ctx.enter_context(tc.tile_pool(name="ys", bufs=3))

        # PSUM: 2+2+1+2+1 = 8 banks
        pacc = ctx.enter_context(tc.tile_pool(name="accpsum", bufs=2, space="PSUM"))
        psco = ctx.enter_context(tc.tile_pool(name="scpsum", bufs=2, space="PSUM"))
        pov = ctx.enter_context(tc.tile_pool(name="ovpsum", bufs=1, space="PSUM"))
        pyp = ctx.enter_context(tc.tile_pool(name="yppsum", bufs=2, space="PSUM"))
        psm = ctx.enter_context(tc.tile_pool(name="smpsum", bufs=1, space="PSUM"))

        def xt_fetch(mc, split_first=False):
            """DMA chunk mc's x tiles (NG groups of GD d-tiles) on sync."""
            grp = []
            for g in range(NG):
                t = pxt.tile([128, GD * 512], bf16, tag="xt", name=f"xt{mc}_{g}")
                r0 = (mc * NG + g) * 128
                if split_first and g == 0:
                    h = GD * 512 // 2
                    nc.sync.dma_start(t[:, 0:h], xT_p[r0:r0 + 128, 0:h])
                    nc.sync.dma_start(t[:, h:], xT_p[r0:r0 + 128, h:])
                else:
                    nc.sync.dma_start(t[:], xT_p[r0:r0 + 128, :])
                grp.append(t)
            return grp

        def cs_fetch(mc, eng):
            t = pcs.tile([128, 4 * 512], bf16, tag="cs", name=f"cs{mc}")
            eng.dma_start(t[:], cs_p[mc * 128:(mc + 1) * 128, :])
            return t

        xt_cur = xt_fetch(0, split_first=True)
        cs_cur = cs_fetch(0, nc.gpsimd)

        for mc in range(MC):
            msl = slice(mc * 512, (mc + 1) * 512)

            qbf = [None] * QH

            for oi, o in enumerate(O_ORDER):
                acc = pacc.tile([128, 512], f32, tag="acc", name=f"acc{mc}_{o}")
                for d in range(DT):
                    woff = (o * DT + d) * 128
                    nc.tensor.matmul(
                        acc[:],
                        w_sb[:, woff:woff + 128],
                        xt_cur[d // GD][:, (d % GD) * 512:(d % GD + 1) * 512],
                        start=(d == 0),
                        stop=(d == DT - 1),
                    )
                # PSUM -> bf16 cast, alternating DVE / ACT(Copy: no table)
                qkv = pqk.tile([128, 512], bf16, tag=f"qk{o}", name=f"qk{o}")
                if oi % 2 == 0:
                    nc.vector.tensor_copy(qkv[:], acc[:])
                else:
                    nc.scalar.activation(qkv[:], acc[:], AF.Copy)

                if o == QH + 1:
                    # V: transpose to [seq, hd] blocks
                    for k in range(4):
                        tp = psm.tile([128, 128], bf16, tag="sm", name="tp")
                        nc.tensor.transpose(
                            tp[:], qkv[:, k * 128:(k + 1) * 128], ident_bf
                        )
                        nc.vector.tensor_copy(v_c[mc][:, k * 128:(k + 1) * 128], tp[:])
                    continue

                # ---- RMS stats ----
                sq = psq.tile([128, 512], bf16, tag="sq", name="sq")
                nc.vector.tensor_mul(sq[:], qkv[:], qkv[:])
                ss = psm.tile([1, 512], f32, tag="sm", name="ss", padded_shape=[128, 512])
                nc.tensor.matmul(ss[:], ones_col_bf, sq[:], start=True, stop=True)
                # bounce ss through SBUF on the DVE so the shared psum bank
                # frees immediately (Sqrt sits behind attention Exps + table
                # loads on ACT; draining via ACT stalls the next bank user)
                ssb = prs.tile([1, 512], f32, tag="ssb", name="ssb")
                nc.vector.tensor_copy(ssb[:], ss[:])
                # for q, sqrt(ss + HD*eps) = sqrt(HD)*sqrt(ms+eps): the recip
                # folds the attention 1/sqrt(HD) scale into q's normalization
                rsq = prs.tile([1, 512], f32, tag="rsq", name="rsq")
                if o < QH:
                    nc.scalar.activation(rsq[:], ssb[:], AF.Sqrt, bias=epsq[:], scale=1.0)
                else:
                    nc.scalar.activation(rsq[:], ssb[:], AF.Sqrt, bias=epsk[:], scale=1.0 / HD)
                nc.vector.reciprocal_approx_fast(out=rsq[:], in_=rsq[:])
                rrb = prb.tile([128, 512], f32, tag="rrb", name="rrb")
                nc.gpsimd.partition_broadcast(rrb[:], rsq[:], channels=128)

                # ---- RoPE ----
                # swap halves via SBUF->SBUF DMA (no PE permute)
                sw = psw.tile([128, 512], bf16, tag="sw", name="sw")
                nc.gpsimd.dma_start(sw[0:HALF, :], qkv[HALF:128, :])
                nc.gpsimd.dma_start(sw[HALF:128, :], qkv[0:HALF, :])
                csb = 0 if o < QH else 2
                t1 = pt.tile([128, 512], bf16, tag="t1", name="t1")
                nc.vector.tensor_mul(t1[:], qkv[:], cs_cur[:, csb * 512:(csb + 1) * 512])
                t2 = pt.tile([128, 512], bf16, tag="t2", name="t2")
                nc.vector.tensor_mul(t2[:], sw[:], cs_cur[:, (csb + 1) * 512:(csb + 2) * 512])
                nc.vector.tensor_add(t1[:], t1[:], t2[:])
                if o < QH:
                    qb = pqb.tile([128, 512], bf16, tag=f"qb{o}", name=f"qb{o}")
                    nc.vector.tensor_mul(qb[:], t1[:], rrb[:])
                    qbf[o] = qb
                else:
                    nc.vector.tensor_mul(kt_c[mc][:], t1[:], rrb[:])

            # next chunk's x/cs prefetch: issued after this chunk's o-passes
            # (sync queue), so slot-release waits can't delay anything else
            if mc + 1 < MC:
                xt_nxt = xt_fetch(mc + 1)
                cs_nxt = cs_fetch(mc + 1, nc.sync)

            # ---------------- attention for chunk mc ----------------
            entries = plan[mc]
            n_ent = len(entries)
            attnT = []
            for h in range(QH):
                # last chunk: no next projection, so its freed acc banks
                # double-buffer ov -> head chains overlap, shorter tail
                if mc == MC - 1 and h % 2 == 1:
                    ov = pacc.tile([128, 512], f32, tag="acc", name="ov")
                else:
                    ov = pov.tile([128, 512], f32, tag="ov", name="ov")
                den = psm.tile([1, 512], f32, tag="sm", name="den", padded_shape=[128, 512])
                for i, (nt, w0, w1, mops) in enumerate(entries):
                    kc, ko = nt // 4, (nt % 4) * 128
                    sc = psco.tile([128, 512], f32, tag="sc", name="sc")
                    nc.tensor.matmul(
                        sc[:, w0:w1], kt_c[kc][:, ko:ko + 128],
                        qbf[h][:, w0:w1],
                        start=True, stop=(not mops),
                    )
                    # masks as -30 score biases (biasT.T @ I accumulated in
                    # PSUM): keeps the whole mask on the PE, no gpsimd/DVE in
                    # the exp chain; exp(s-30) ~ 5e-9 is negligible in den/ov
                    for bi, (j, kind, tix) in enumerate(mops):
                        jsl = slice(j * 128, (j + 1) * 128)
                        bias = tri_bf if kind == "tri" else mask_bf[:, tix * 128:(tix + 1) * 128]
                        nc.tensor.matmul(
                            sc[:, jsl], bias, ident_bf,
                            start=False, stop=(bi == len(mops) - 1),
                            skip_group_check=True,
                        )
                    ex = pex.tile([128, 512], bf16, tag="ex", name="ex")
                    nc.scalar.activation(ex[:, w0:w1], sc[:, w0:w1], AF.Exp)
                    first = i == 0
                    last = i == n_ent - 1
                    nc.tensor.matmul(
                        ov[:, w0:w1], v_c[kc][:, ko:ko + 128], ex[:, w0:w1],
                        start=first, stop=last, skip_group_check=True,
                    )
                    nc.tensor.matmul(
                        den[0:1, w0:w1], ones_col_bf, ex[:, w0:w1],
                        start=first, stop=last, skip_group_check=True,
                    )
                den_sb = pdn.tile([1, 512], f32, tag="den_sb", name="den_sb")
                nc.vector.tensor_copy(den_sb[:], den[:])
                nc.vector.reciprocal_approx_fast(out=den_sb[:], in_=den_sb[:])
                rrb2 = prb.tile([128, 512], f32, tag="rrb2", name="rrb2")
                nc.gpsimd.partition_broadcast(rrb2[:], den_sb[:], channels=128)
                at = pat.tile([128, 512], bf16, tag=f"attnT{h}", name=f"attnT{h}")
                nc.vector.tensor_mul(at[:], ov[:], rrb2[:])
                attnT.append(at)

            # ---------------- output projection for chunk mc ----------------
            ypn = [0]

            def yp_group(jsl, ec, ts, start, stop):
                # last chunk: the idle ov bank becomes a third outproj slot,
                # hiding the cast-drain latency with no next-chunk filler
                ypn[0] += 1
                if mc == MC - 1 and ypn[0] % 3 == 0:
                    yp = pov.tile([128, 512], f32, tag="ov", name="yp")
                else:
                    yp = pyp.tile([128, 512], f32, tag="yp", name="yp")
                for ti, t in enumerate(ts):
                    nc.tensor.matmul(
                        yp[:],
                        attnT[t][:, jsl],
                        wo_sb[:, t * D + ec * 512: t * D + (ec + 1) * 512],
                        start=(start and ti == 0),
                        stop=(stop and ti == len(ts) - 1),
                    )
                return yp

            for j in range(4):
                mt = mc * 4 + j
                tsl = slice(mt * 128, (mt + 1) * 128)
                jsl = slice(j * 128, (j + 1) * 128)
                # last chunk, first two j-tiles: split the head contraction so
                # half the outproj matmuls are ready after head 1 and overlap
                # the remaining attention chains (no next-chunk filler exists)
                split = mc == MC - 1 and j < 2
                if split:
                    ys_h = []
                    for half in range(2):
                        ys = pys.tile([128, D // 2], bf16, tag="ys", name="ys")
                        for eh in range(4):
                            ec = half * 4 + eh
                            yp = yp_group(jsl, ec, (0, 1), True, True)
                            esl = slice(eh * 512, (eh + 1) * 512)
                            if ec % 2 == 0:
                                nc.scalar.activation(ys[:, esl], yp[:], AF.Copy)
                            else:
                                nc.vector.tensor_copy(ys[:, esl], yp[:])
                        ys_h.append(ys)
                    for half in range(2):
                        ys = ys_h[half]
                        for eh in range(4):
                            ec = half * 4 + eh
                            yp = yp_group(jsl, ec, (2, 3), True, True)
                            esl = slice(eh * 512, (eh + 1) * 512)
                            nc.vector.scalar_tensor_tensor(
                                ys[:, esl], yp[:], 1.0, ys[:, esl],
                                mybir.AluOpType.mult, mybir.AluOpType.add,
                            )
                        h0 = half * (D // 2)
                        nc.sync.dma_start(out_p[tsl, h0:h0 + D // 2], ys[:])
                else:
                    for half in range(2):
                        ys = pys.tile([128, D // 2], bf16, tag="ys", name="ys")
                        for eh in range(4):
                            ec = half * 4 + eh
                            yp = yp_group(jsl, ec, range(QH), True, True)
                            esl = slice(eh * 512, (eh + 1) * 512)
                            if ec % 2 == 0:
                                nc.scalar.activation(ys[:, esl], yp[:], AF.Copy)
                            else:
                                nc.vector.tensor_copy(ys[:, esl], yp[:])
                        h0 = half * (D // 2)
                        nc.sync.dma_start(out_p[tsl, h0:h0 + D // 2], ys[:])

            if mc + 1 < MC:
                xt_cur = xt_nxt
                cs_cur = cs_nxt

    nc.finalize()
    return nc


def kernel(x, wq, wk, wv, wo, q_norm_w, k_norm_w, rope_cache, positions, cu_seqlens):
    global LAST_RESULT
    from concourse.bass_utils import run_bass_kernel_spmd  # noqa: PLC0415

    x = np.asarray(x, dtype=np.float32)
    wq = np.asarray(wq, dtype=np.float32)
    wk = np.asarray(wk, dtype=np.float32)
    wv = np.asarray(wv, dtype=np.float32)
    wo = np.asarray(wo, dtype=np.float32)
    q_norm_w = np.asarray(q_norm_w, dtype=np.float32)
    k_norm_w = np.asarray(k_norm_w, dtype=np.float32)
    rope_cache = np.asarray(rope_cache, dtype=np.float32)
    positions = np.asarray(positions)
    cu_seqlens = np.asarray(cu_seqlens)

    import ml_dtypes  # noqa: PLC0415

    bf = ml_dtypes.bfloat16

    # ---- host prep (shared) ----
    # x pretiled into (mc, g) groups of GD d-tiles: group (mc,g) = rows
    # [(mc*NG+g)*128, +128), cols di*512+c  <->  xT[(g*GD+di)*128+p, mc*512+c]
    xT = x[0].T.astype(bf)                       # [D, S]
    xt_host = np.ascontiguousarray(
        xT.reshape(NG, GD, 128, MC, 512).transpose(3, 0, 2, 1, 4)
        .reshape(MC * NG * 128, GD * 512)
    )

    pos = positions.reshape(-1)
    cs = rope_cache[pos]               # [S, HALF, 2]
    cosT = cs[:, :, 0].T               # [HALF, S]
    sinT = cs[:, :, 1].T
    cs1 = np.concatenate([cosT, cosT], axis=0)    # [128, S]
    cs2 = np.concatenate([-sinT, sinT], axis=0)

    def fold(w):
        w = w.reshape(HD, 1)
        wsw = np.concatenate([w[HALF:], w[:HALF]], axis=0)
        return cs1 * w, cs2 * wsw

    cs1q, cs2q = fold(q_norm_w)
    cs1k, cs2k = fold(k_norm_w)
    # per-chunk [128, 4*512]: rows mc*128+p, block ci at cols ci*512
    cs_host = np.ascontiguousarray(
        np.stack([cs1q, cs2q, cs1k, cs2k], axis=0).astype(bf)
        .reshape(4, 128, MC, 512).transpose(2, 1, 0, 3)
        .reshape(MC * 128, 4 * 512)
    )

    plan, mask_pack = _attention_plan(cu_seqlens)

    consts_bf = np.zeros((128, 3 * 128 + mask_pack.shape[1]), dtype=np.float32)
    consts_bf[:, 0:128] = 1.0
    consts_bf[:, 128:256] = np.eye(128, dtype=np.float32)
    # causal bias lhsT[m, n] = -30 where m < n (strictly upper)
    consts_bf[:, 256:384] = np.triu(np.full((128, 128), -30.0), k=1)
    consts_bf[:, 384:] = mask_pack
    consts_bf = consts_bf.astype(bf)

    # ---- per-core weight shards ----
    in_maps = []
    for c in range(NCORES):
        w_all = np.concatenate(
            [
                wq[c * QH * HD:(c + 1) * QH * HD],   # [512, D]
                wk[c * HD:(c + 1) * HD],             # [128, D]
                wv[c * HD:(c + 1) * HD],             # [128, D]
            ],
            axis=0,
        )  # [NO*128, D]
        # per-o pretile: block o = rows [o*128, +128) holding [128(dd), DT*128]
        # with (o, d) stationary lhsT[dd, ff] = w_all[o*128+ff, d*128+dd]
        w_host = np.ascontiguousarray(
            w_all.reshape(NO, 128, DT, 128).transpose(0, 3, 2, 1)
            .reshape(NO * 128, DT * 128).astype(bf)
        )
        wo_c = wo[:, c * QH * HD:(c + 1) * QH * HD].T  # [512, D]
        wo_host = np.ascontiguousarray(
            wo_c.reshape(QH, 128, D).transpose(1, 0, 2)
            .reshape(128, 4, QH * D // 4).transpose(1, 0, 2)
            .reshape(4 * 128, QH * D // 4).astype(bf)
        )
        in_maps.append(
            {
                "xT": xt_host,
                "w_qkv": w_host,
                "w_o": wo_host,
                "cs": cs_host,
                "consts_bf": consts_bf,
            }
        )

    nc = _build_graph(plan, mask_pack.shape[1])
    res = run_bass_kernel_spmd(nc, in_maps, list(range(NCORES)))
    LAST_RESULT = res

    out = res.results[0]["out"].astype(np.float32)
    for c in range(1, NCORES):
        out = out + res.results[c]["out"].astype(np.float32)
    return out.reshape(1, S, D)
